# revision 1
# baseline (speedup 1.0000x reference)
"""Trainium2 Bass kernel for InvariantMessagePassingTP.

out[n, lm, c] = sum_{e: recv[e]=n} edge_attrs[e,lm] * tp_weights[e,l(lm),c]
                * node_feats[recv[e], c]

Strategy (8 NeuronCores, SPMD, no collectives):
  receiver_list is sorted -> each core owns a contiguous node range (3125
  nodes) and its contiguous edge range. The host greedily groups nodes into
  "tiles": <=8 nodes and <=128 edges per tile. Edges sit on SBUF partitions.

  Per tile (the A-fold trick - both A and the one-hot scatter live in the
  matmul stationary):
    U[e, l*64+c]      = W[e,l,c] * F[e,c]          (DVE TT bf16 2x, batched)
    At[e, lm*8+k]     = A[e,lm]  * S8[e,k]         (DVE TT bf16 2x;
                        S8 = one-hot of the node's local index k in 0..7)
    P = At^T @ U      (PE, one matmul N=256, fp32 PSUM: P[lm*8+k, l*64+c]
                       = sum_e A*S8*W*F -- rows (lm,k), col block l(lm)
                       holds the answer)
  8 tiles share one PSUM tile; ACT then copies each l-column-block of PSUM
  (all 128 lanes) to bf16 staging, and per-l DMAs ship only the valid row
  ranges to DRAM laid out as slots[lm, k, tile, c]. The host gathers
  slots -> out[node, lm, c] (summing in the rare case a node spans tiles).
"""

import sys

sys.path.insert(0, "/opt/trn_rl_repo")

import numpy as np
import ml_dtypes

import concourse.bass as bass
import concourse.bacc as bacc
import concourse.tile as tile
from concourse import mybir
from concourse.bass_utils import run_bass_kernel_spmd

NPBF = ml_dtypes.bfloat16
BF16 = mybir.dt.bfloat16
F32 = mybir.dt.float32

NNODES = 25000
NEDGES = 400000
NCHAN = 64
N_CORES = 8
NPC = NNODES // N_CORES        # nodes per core
TB = 360                       # bf16 elems per tile per partition
CHUNK = 32                     # tiles per input DMA chunk
PSB = 8                        # tiles per PSUM batch
MSG_B = 4                      # tiles per U-op batch

L_OF_LM = np.array([0, 1, 1, 1, 2, 2, 2, 2, 2, 3, 3, 3, 3, 3, 3, 3], np.int64)
L_GROUPS = [(0, 1), (1, 3), (4, 5), (9, 7)]  # (lm_start, m_l) for l=0..3
# row-block order of lm in At / PSUM / slots: l2,l3 first (96 rows at psum
# base 0), then l0,l1 (32 rows at base 96) - matmul psum-base constraint.
PERM_LM = [4, 5, 6, 7, 8, 9, 10, 11, 12, 13, 14, 15, 0, 1, 2, 3]

_PROGRAM_CACHE = {}


def _greedy_groups(deg, node0):
    """Group consecutive nodes: <=8 nodes, <=128 edges per group.
    A node with deg>128 is split across several single-node groups.
    Returns list of (node_start, n_nodes, n_edges_in_group) with node-split
    groups flagged by n_nodes==1 repeats."""
    groups = []
    n = len(deg)
    i = 0
    while i < n:
        if deg[i] > 128:
            # split this node's edges over several groups
            rem = deg[i]
            while rem > 0:
                take = min(128, rem)
                groups.append((node0 + i, 1, take))
                rem -= take
            i += 1
            continue
        cnt = 0
        edges = 0
        while i + cnt < n and cnt < 8 and edges + deg[i + cnt] <= 128:
            edges += deg[i + cnt]
            cnt += 1
        groups.append((node0 + i, cnt, edges))
        i += cnt
    return groups


def _build_schedule(receiver_list):
    recv = np.asarray(receiver_list).astype(np.int64)
    deg = np.bincount(recv, minlength=NNODES)
    per_core = []
    for c in range(N_CORES):
        per_core.append(_greedy_groups(deg[c * NPC:(c + 1) * NPC], c * NPC))
    t_max = max(len(g) for g in per_core)
    t_u = -(-t_max // PSB) * PSB  # round up to PSUM batch
    return recv, deg, per_core, t_u


def _pack_inputs(node_feats, edge_attrs, tp_weights, recv, per_core, t_u):
    w_bf = np.asarray(tp_weights, np.float32).reshape(NEDGES, 256).astype(NPBF)
    f_bf = np.asarray(node_feats, np.float32).astype(NPBF)
    a_bf = np.asarray(edge_attrs, np.float32).astype(NPBF)
    # edge start index of each node (recv sorted)
    node_e0 = np.searchsorted(recv, np.arange(NNODES + 1))

    in_maps = []
    slot_maps = []  # per core: list of (node_start, n_nodes) per tile
    for c in range(N_CORES):
        groups = per_core[c]
        T = t_u
        # slot-major staging [T*128, TB]:
        # [ W 0:256 | F 256:320 | A2 320:352 | S8 352:360 ]
        X = np.zeros((T * 128, TB), NPBF)
        smap = []
        e_cursor = {}
        for t, (n0, k, ne) in enumerate(groups):
            if ne == 0:
                smap.append((n0, k))
                continue
            e0 = node_e0[n0] + e_cursor.get(n0, 0) if k == 1 else node_e0[n0]
            # for split nodes track consumed edges
            if k == 1:
                e_cursor[n0] = e_cursor.get(n0, 0) + ne
            e1 = e0 + ne
            base = t * 128
            X[base:base + ne, 0:256] = w_bf[e0:e1]
            X[base:base + ne, 256:320] = f_bf[recv[e0:e1]]
            a2 = np.repeat(a_bf[e0:e1][:, PERM_LM], 2, axis=1)
            X[base:base + ne, 320:352] = a2
            loc = (recv[e0:e1] - n0).astype(np.int64)  # 0..7
            X[base + np.arange(ne), 352 + loc] = NPBF(1.0)
            smap.append((n0, k))
        while len(smap) < T:
            smap.append((0, 0))
        # chunk-block-major device layout
        Xt = X.reshape(T, 128, TB)
        n_chunks = -(-T // CHUNK)
        buf = np.zeros((128, T * TB), NPBF)
        pos = 0
        for ch in range(n_chunks):
            t0, t1 = ch * CHUNK, min((ch + 1) * CHUNK, T)
            for so, sz in ((0, 256), (256, 64), (320, 32), (352, 8)):
                blk = Xt[t0:t1, :, so:so + sz]  # [ct, 128, sz]
                ct = t1 - t0
                buf[:, pos:pos + ct * sz] = (
                    blk.transpose(1, 0, 2).reshape(128, ct * sz))
                pos += ct * sz
        in_maps.append({"inp": buf})
        slot_maps.append(smap)
    return in_maps, slot_maps


def _build_program(t_u):
    nc = bacc.Bacc("TRN2", target_bir_lowering=False, debug=False,
                   num_devices=N_CORES)
    T = t_u
    in_d = nc.dram_tensor("inp", [128, T * TB], BF16, kind="ExternalInput").ap()
    # slots[row = perm-lm-block*8 + k, tile, c]
    out_d = nc.dram_tensor("out", [128, T, 64], BF16,
                           kind="ExternalOutput").ap()

    n_chunks = -(-T // CHUNK)
    with tile.TileContext(nc) as tc:
        with tc.tile_pool(name="ld", bufs=3) as ld_pool, \
             tc.tile_pool(name="u", bufs=6) as u_pool, \
             tc.tile_pool(name="at", bufs=20) as at_pool, \
             tc.tile_pool(name="st", bufs=3) as st_pool, \
             tc.tile_pool(name="ps", bufs=4, space="PSUM") as ps_pool:
            for ch in range(n_chunks):
                t0, t1 = ch * CHUNK, min((ch + 1) * CHUNK, T)
                ct = t1 - t0
                # chunk block offsets (bf16 elems within the chunk)
                oW, oF, oA, oS = 0, ct * 256, ct * 320, ct * 352
                base_el = t0 * TB
                ld = ld_pool.tile([128, ct * TB], BF16, tag="ld")
                nc.sync.dma_start(
                    out=ld,
                    in_=bass.AP(
                        tensor=in_d.tensor, offset=base_el,
                        ap=[[T * TB, 128], [1, ct * TB]]),
                )
                # per-chunk staging: [128, half, ct, 64] bf16
                stage = st_pool.tile([128, 2, ct, 64], BF16, tag="stage")
                for p0 in range(0, ct, PSB):
                    ps = ps_pool.tile([128, PSB, 128], F32, tag="ps")
                    ats = []
                    us = []
                    for b0 in range(p0, p0 + PSB, MSG_B):
                        bn = MSG_B
                        # U = W * F -> [128, bn, 4, 64]
                        u = u_pool.tile([128, MSG_B, 256], BF16, tag="u")
                        us.append(u)
                        w_v = ld[:, oW + b0 * 256: oW + (b0 + bn) * 256]
                        f_v = ld[:, oF + b0 * 64: oF + (b0 + bn) * 64]
                        nc.vector.tensor_mul(
                            u[:, :bn].rearrange("p t (l c) -> p t l c", l=4),
                            w_v.rearrange("p (t l c) -> p t l c", t=bn, l=4),
                            f_v.rearrange("p (t c) -> p t c", t=bn)[
                                :, :, None, :].broadcast_to([128, bn, 4, 64]),
                        )
                        for b in range(bn):
                            t = b0 + b
                            # At[e, lm*8+k] = A2[e,lm,d] * S8[e,k]
                            at = at_pool.tile([128, 128], BF16, tag="at")
                            ats.append(at)
                            a_v = ld[:, oA + t * 32: oA + (t + 1) * 32]
                            s_v = ld[:, oS + t * 8: oS + (t + 1) * 8]
                            nc.vector.tensor_mul(
                                at.rearrange("p (l q d) -> p l q d",
                                             l=16, d=2),
                                a_v.rearrange("p (l d) -> p l d", d=2)[
                                    :, :, None, :].broadcast_to(
                                        [128, 16, 4, 2]),
                                s_v.rearrange("p (q d) -> p q d", d=2)[
                                    :, None, :, :].broadcast_to(
                                        [128, 16, 4, 2]),
                            )
                    # phase A: rows 0-95 = (l2|l3) x U cols 128:256
                    for k in range(PSB):
                        nc.tensor.matmul(
                            ps[0:96, k], ats[k][:, 0:96],
                            us[k // MSG_B][:, k % MSG_B, 128:256],
                            start=True, stop=True)
                    # phase B: rows 96-127 = (l0|l1) x U cols 0:128
                    for k in range(PSB):
                        nc.tensor.matmul(
                            ps[96:128, k], ats[k][:, 96:128],
                            us[k // MSG_B][:, k % MSG_B, 0:128],
                            start=True, stop=True,
                            tile_position=(0, 96))
                    # full-lane extraction of the whole PSUM batch into the
                    # chunk stage, col halves separated for contiguous DMA
                    nc.scalar.copy(
                        bass.AP(
                            tensor=stage.tensor, offset=stage.offset + p0 * 64,
                            ap=[stage.ap[0], [64, PSB], [ct * 64, 2],
                                [1, 64]]),
                        ps,
                    )
                # 4 out-DMA fragments per chunk; DMA picks valid rows
                for (r0, r1, half) in ((0, 40, 0), (40, 96, 1),
                                       (96, 104, 0), (104, 128, 1)):
                    nc.sync.dma_start(
                        out=bass.AP(
                            tensor=out_d.tensor,
                            offset=r0 * (T * 64) + t0 * 64,
                            ap=[[T * 64, r1 - r0], [64, ct], [1, 64]]),
                        in_=stage[r0:r1, half],
                    )
    nc.compile()
    return nc


def kernel(node_feats, edge_attrs, tp_weights, receiver_list, nnodes,
           _trace=False):
    node_feats = np.asarray(node_feats)
    edge_attrs = np.asarray(edge_attrs)
    tp_weights = np.asarray(tp_weights)
    receiver_list = np.asarray(receiver_list)
    nnodes = int(nnodes)
    assert node_feats.shape == (NNODES, NCHAN) and nnodes == NNODES
    assert tp_weights.shape == (NEDGES, 4, NCHAN)

    recv, deg, per_core, t_u = _build_schedule(receiver_list)
    key = int(t_u)
    if key not in _PROGRAM_CACHE:
        _PROGRAM_CACHE[key] = _build_program(t_u)
    nc = _PROGRAM_CACHE[key]

    in_maps, slot_maps = _pack_inputs(
        node_feats, edge_attrs, tp_weights, recv, per_core, t_u)
    res = run_bass_kernel_spmd(nc, in_maps, list(range(N_CORES)),
                               trace=_trace)

    inv = np.argsort(np.array(PERM_LM))  # lm -> row-block index
    out = np.zeros((NNODES, 16, NCHAN), np.float32)
    for c in range(N_CORES):
        slots = res.results[c]["out"].astype(np.float32)  # [128, T, 64]
        slots = slots.reshape(16, 8, -1, NCHAN)[inv]  # [lm, k, T, c]
        smap = slot_maps[c]
        for t, (n0, k) in enumerate(smap):
            if k == 0:
                continue
            out[n0:n0 + k] += slots[:, 0:k, t, :].transpose(1, 0, 2)
    if _trace:
        return out, res
    return out

